# revision 2
# baseline (speedup 1.0000x reference)
"""Multi-head causal attention (B=2, S=2048, D=1024, H=16, Dh=64) on 8 TRN2
NeuronCores.

Sharding: core c = 4*b + g handles batch b (2-way data parallel) and head
group g (4-way tensor parallel: heads 4g..4g+3 = a 256-column slice of
W_q/W_k/W_v and the matching 256-row slice of W_o). Each core returns a
partial output [S, D] fp16; the host sums the 4 partials per batch and adds
b_o.

Precision/speed strategy (cost-model driven):
- Q/K/V projections: scaled hi/lo-split fp8e4 with DoubleRow matmuls
  (256-deep contraction at 0.5 cyc/col = 4x bf16): s*W = wh + wl,
  x = xh + xl (host-side splits; the scale s keeps wh in e4m3's normal
  range). One 3-term PSUM chain xh@wh + xh@wl + xl@wh = s*(x@W) at ~0.2%
  error and 0.75x the bf16 PE cost.
- Scores: fp8 DoubleRow with the Dh=64 contraction folded as [32, 2]
  (folded Q/K built by SB->SB DMAs; head h at partition base PB[h],
  column base CB[h]). Scales fold into the exp's scale operand.
- e/V/ct/W_o in fp16. V carries scale SV and the softmax-denominator
  ones-columns hold SV, so the normalize ratio is exact.
- K-major flash attention: scores transposed (S^T[k,q]) so the softmax
  denominator folds into the attn@V matmul via ones-columns in V.
- exp is scalar-engine-only and is the pacing engine (~58us of width +
  per-instruction overhead): k-tiles are processed in PAIRS sharing one
  [128,2048] PSUM tile and ONE exp instruction; scores of the next
  (head, q-chunk) are prefetched during the current chunk's attn@V so the
  exp stream never starves. Everything else is kept off the scalar engine
  (DVE copies, gpsimd masks) until the post-exp tail.
- DMAs coalesced (each dma_start serializes ~640ns on the shared HWDGE).
"""

import numpy as np
import ml_dtypes
from contextlib import ExitStack

import concourse.bass as bass
import concourse.bacc as bacc
import concourse.tile as tile
import concourse.mybir as mybir
from concourse.bass_utils import run_bass_kernel_spmd

F32 = mybir.dt.float32
F16 = mybir.dt.float16
F8 = mybir.dt.float8e4
AF = mybir.ActivationFunctionType
DR = mybir.MatmulPerfMode.DoubleRow
E4 = ml_dtypes.float8_e4m3

B = 2
S = 2048
D = 1024
DC = 256  # head dims per core (4 heads x 64)
N_CORES = 8
NT = D // 128  # 8 input-dim tiles
ST = S // 128  # 16 sequence tiles

SQ = 512.0  # fp8 scale on W_q (incl. 1/sqrt(Dh) fold) and so on Q
SK = 64.0   # fp8 scale on W_k / K
SV = 64.0   # fp8 scale on W_v / V (fp16 V and ones-columns carry SV)


def _slices512(off, end):
    out = []
    a = off
    while a < end:
        b = min(end, (a // 512 + 1) * 512)
        out.append((a, b))
        a = b
    return out


def _build():
    nc = bacc.Bacc("TRN2", target_bir_lowering=False, debug=False,
                   num_devices=N_CORES)
    xh = nc.dram_tensor("xh", [128, NT * S], F8, kind="ExternalInput").ap()
    xl = nc.dram_tensor("xl", [128, NT * S], F8, kind="ExternalInput").ap()
    wts = {}
    for t in ("q", "k", "v"):
        for p in ("h", "l"):
            wts[t + p] = nc.dram_tensor(
                f"w{t}{p}", [128, NT * DC], F8, kind="ExternalInput").ap()
    wo = nc.dram_tensor("wo", [DC, D], F16, kind="ExternalInput").ap()
    mk = nc.dram_tensor("mk", [128, 128], F16, kind="ExternalInput").ap()
    y = nc.dram_tensor("y", [S, D], F16, kind="ExternalOutput").ap()

    with tile.TileContext(nc) as tc, ExitStack() as stk:
        persist = stk.enter_context(tc.tile_pool(name="persist", bufs=1))
        # folded Q^T/K^T: head h at partitions PB[h]..PB[h]+32, column base
        # CB[h]; fold slot i (dims 32i..32i+32) in the middle dim. (h3 goes
        # to partitions 0 at +S cols: matmul partition base must be 0/32/64.)
        qf = persist.tile([128, 2, 2 * S], F8)
        kf = persist.tile([128, 2, 2 * S], F8)
        # V per k-tile block of 512 cols: head h sub-block of 128 cols =
        # [SV*V_h | SV*ones] for even h, [SV*ones | SV*V_h] for odd h.
        v_sb = persist.tile([128, ST * 512], F16)
        ct_sb = persist.tile([128, 2 * S], F16)   # normalized ctx^T
        wo_sb = persist.tile([128, 2, D], F16)
        mk_sb = persist.tile([128, 128], F16)     # mask[k, q] = (k <= q)
        xh_sb = persist.tile([128, NT, S], F8)
        xl_sb = persist.tile([128, NT, S], F8)
        w_sb = {n: persist.tile([128, NT, DC], F8, name=f"w_{n}")
                for n in ("qh", "ql", "kh", "kl", "vh", "vl")}

        # preload the Exp activation table off the critical path
        warm = persist.tile([128, 8], F32)
        nc.vector.memset(warm[:, :], 0.0)
        nc.scalar.activation(warm[:, :], warm[:, :], AF.Exp)
        # ones-columns of V (cols 64:192 of each 256-block), value SV, once
        nc.vector.memset(
            v_sb[:, :].rearrange("p (b c) -> p b c", c=256)[:, :, 64:192], SV)

        # ---- input DMAs (host pre-tiled; x by column-halves, emitted in
        # need-order: the shared DMA device serves transfers in issue
        # order, so non-critical loads must come AFTER the j0 folds) ----
        def x_chunk(xsb, xd, hf):
            c0, c1 = 1024 * hf, 1024 * (hf + 1)
            nc.sync.dma_start(
                out=xsb[:, :, c0:c1],
                in_=xd.rearrange("p (i c) -> p i c", c=S)[:, :, c0:c1])

        def w_load(n):
            nc.sync.dma_start(
                out=w_sb[n][:, :, :],
                in_=wts[n].rearrange("p (i c) -> p i c", c=DC))

        for n in ("kh", "qh"):
            w_load(n)
        x_chunk(xh_sb, xh, 0)
        for n in ("kl", "ql"):
            w_load(n)
        x_chunk(xl_sb, xl, 0)
        nc.sync.dma_start(out=mk_sb[:], in_=mk[:, :])

        ep = stk.enter_context(tc.tile_pool(name="ep", bufs=34))
        rp = stk.enter_context(tc.tile_pool(name="rp", bufs=2))
        stg = stk.enter_context(tc.tile_pool(name="stg", bufs=2))
        ob = stk.enter_context(tc.tile_pool(name="ob", bufs=2))

        PB = (0, 32, 64, 0)       # folded partition base per head
        CB = (0, 0, 0, S)         # folded column base per head

        e_tiles = {}  # (h, qc, kt) -> (e_sb [128,1024], off)

        def emit_scores_exp(h, qc, kt, pool):
            q0 = 1024 * qc
            pb, cb = PB[h], CB[h]
            off = max(q0, 128 * kt) - q0
            s_ps = pool.tile([128, 1024], F32, tag="s", name=f"s{h}{qc}{kt}")
            for a, b in _slices512(off, 1024):
                nc.tensor.matmul(
                    s_ps[:, a:b],
                    lhsT=kf[pb:pb + 32, :,
                            cb + 128 * kt:cb + 128 * (kt + 1)],
                    rhs=qf[pb:pb + 32, :, cb + q0 + a:cb + q0 + b],
                    start=True, stop=True, perf_mode=DR)
            e_sb = ep.tile([128, 1024], F16, tag="e", name=f"e{h}{qc}{kt}")
            nc.scalar.activation(e_sb[:, off:1024], s_ps[:, off:1024],
                                 AF.Exp, scale=1.0 / (SQ * SK))
            if 128 * kt >= q0:
                # diagonal block: zero strictly-lower (k > q); DVE fp16 2x
                nc.vector.tensor_mul(e_sb[:, off:off + 128],
                                     e_sb[:, off:off + 128], mk_sb[:, :])
            e_tiles[(h, qc, kt)] = (e_sb, off)

        def emit_ctx(h, qc, kt, ctx_ps):
            e_sb, off = e_tiles.pop((h, qc, kt))
            for a, b in _slices512(off, 1024):
                last_kt = 8 * qc + (3 if b <= 512 else 7)
                nc.tensor.matmul(
                    ctx_ps[:, a:b],
                    lhsT=v_sb[:, 512 * kt + 128 * h:
                              512 * kt + 128 * (h + 1)],
                    rhs=e_sb[:, a:b],
                    start=(kt == 0), stop=(kt == last_kt))

        def emit_norm(h, qc, ctx_ps):
            jh = h // 2
            hb = 64 * (h % 2)
            dr = 64 - hb
            rcp = rp.tile([128, 1024], F32, tag="rcp", name=f"r{h}{qc}")
            rcb = rp.tile([128, 1024], F32, tag="rcb", name=f"rb{h}{qc}")
            # NB: reciprocal_approx_* miscompute at partition base != 0
            nc.vector.reciprocal(rcp[dr:dr + 1, :], ctx_ps[dr:dr + 1, :])
            if dr == 0:
                nc.gpsimd.partition_broadcast(rcb[:, :], rcp[0:1, :])
            else:
                nc.sync.dma_start(
                    out=rcb[hb:hb + 64, :],
                    in_=rcp[dr:dr + 1, :].unsqueeze(1)
                    .to_broadcast((1, 64, 1024)))
            nc.vector.tensor_mul(
                ct_sb[hb:hb + 64,
                      2048 * jh + 1024 * qc:2048 * jh + 1024 * (qc + 1)],
                ctx_ps[hb:hb + 64, :], rcb[hb:hb + 64, :])

        # ---- phase A: projections + h0 scores interleaved ----
        with tc.tile_pool(name="ppq", bufs=2, space="PSUM") as ppq, \
             tc.tile_pool(name="ppv", bufs=2, space="PSUM") as ppv, \
             tc.tile_pool(name="sp1", bufs=2, space="PSUM") as sp1:

            def qk_chain(t, j, half, sl, dst_stg):
                """One [128,512] psum chain: 3-term hi/lo fp8 DR products
                accumulating sW*(x@W); copy out as fp8."""
                ps = ppq.tile([128, 512], F32, tag="ppq",
                              name=f"p{t}{j}{half}{sl}")
                a = 1024 * half + 512 * sl
                jj = slice(128 * j, 128 * (j + 1))
                terms = ((xh_sb, w_sb[t + "h"]), (xh_sb, w_sb[t + "l"]),
                         (xl_sb, w_sb[t + "h"]))
                for ti, (xs, ws) in enumerate(terms):
                    for p in range(4):
                        nc.tensor.matmul(
                            ps[:, :],
                            lhsT=ws[:, 2 * p:2 * p + 2, jj],
                            rhs=xs[:, 2 * p:2 * p + 2, a:a + 512],
                            start=(ti == 0 and p == 0),
                            stop=(ti == 2 and p == 3), perf_mode=DR)
                nc.vector.tensor_copy(dst_stg[:, a:a + 512], ps[:, :])

            def fold(j, half, src_stg, dst):
                """Staging half [128,1024] -> folded layout: head 2j+u,
                dims 32i:32i+32 -> dst[PB[h]:+32, i, CB[h]+1024*half:]."""
                a = 1024 * half
                for u in range(2):
                    h = 2 * j + u
                    pb, cb = PB[h], CB[h]
                    for i in range(2):
                        nc.sync.dma_start(
                            out=dst[pb:pb + 32, i, cb + a:cb + a + 1024],
                            in_=src_stg[64 * u + 32 * i:
                                        64 * u + 32 * (i + 1), a:a + 1024])

            def v_round(st):
                pv = ppv.tile([128, 256], F32, tag="ppv", name=f"pv{st}")
                for ti, (xs, ws) in enumerate(
                        ((xh_sb, w_sb["vh"]), (xh_sb, w_sb["vl"]),
                         (xl_sb, w_sb["vh"]))):
                    for p in range(4):
                        nc.tensor.matmul(
                            pv[:, :],
                            lhsT=xs[:, 2 * p:2 * p + 2,
                                    128 * st:128 * (st + 1)],
                            rhs=ws[:, 2 * p:2 * p + 2, :],
                            start=(ti == 0 and p == 0),
                            stop=(ti == 2 and p == 3), perf_mode=DR)
                base = 512 * st
                blk = v_sb[:, base:base + 512].rearrange(
                    "p (h c) -> p h c", c=256)
                srcv = pv[:, :].rearrange("p (h c) -> p h c", c=128)
                nc.vector.tensor_copy(blk[:, :, 0:64], srcv[:, :, 0:64])
                nc.vector.tensor_copy(blk[:, :, 192:256], srcv[:, :, 64:128])

            sc0 = [(0, 0, kt) for kt in range(8)] + \
                  [(1, 0, kt) for kt in range(8)]   # narrow: j0 half-0 fold
            sc1 = [(0, 1, kt) for kt in range(16)]  # wide: j0 half-1 fold

            def sc_pop(lst, n):
                for _ in range(min(n, len(lst))):
                    h, qc, kt = lst.pop(0)
                    emit_scores_exp(h, qc, kt, sp1)

            ks = stg.tile([128, 2048], F8, tag="stg", name="ks0")
            qs = stg.tile([128, 2048], F8, tag="stg", name="qs0")
            for sl in range(2):
                qk_chain("k", 0, 0, sl, ks)
            for sl in range(2):
                qk_chain("q", 0, 0, sl, qs)
            fold(0, 0, ks, kf)
            fold(0, 0, qs, qf)
            # narrow h0/h1-qc0 scores only need q/k cols 0:1024 (half 0)
            sc_pop(sc0, 2)
            # non-critical loads AFTER the first folds' device slots
            x_chunk(xh_sb, xh, 1)
            x_chunk(xl_sb, xl, 1)
            for sl in range(2):
                qk_chain("k", 0, 1, sl, ks)
                sc_pop(sc0, 2)
            for sl in range(2):
                qk_chain("q", 0, 1, sl, qs)
                sc_pop(sc0, 2)
            fold(0, 1, ks, kf)
            fold(0, 1, qs, qf)
            for n in ("vh", "vl"):
                w_load(n)
            sc_pop(sc0, 2)
            sc_pop(sc1, 2)
            ks1 = stg.tile([128, 2048], F8, tag="stg", name="ks1")
            qs1 = stg.tile([128, 2048], F8, tag="stg", name="qs1")
            for half in range(2):
                for sl in range(2):
                    qk_chain("k", 1, half, sl, ks1)
                    sc_pop(sc1, 1)
            fold(1, 0, ks1, kf)
            fold(1, 1, ks1, kf)
            for half in range(2):
                for sl in range(2):
                    qk_chain("q", 1, half, sl, qs1)
                    sc_pop(sc1, 1)
            fold(1, 0, qs1, qf)
            fold(1, 1, qs1, qf)
            for d in range(2):
                nc.sync.dma_start(out=wo_sb[:, d, :],
                                  in_=wo[128 * d:128 * (d + 1), :])
            for st in range(ST):
                v_round(st)
                sc_pop(sc1 if sc1 else sc0, 1)

        # ---- phase B: attention + out-projection ----
        chunks = [(0, 0), (0, 1), (1, 0), (1, 1),
                  (2, 0), (2, 1), (3, 0), (3, 1)]
        pending = {c: list(range(8 * (c[1] + 1))) for c in chunks
                   if c not in ((0, 0), (0, 1), (1, 0))}

        with tc.tile_pool(name="sp", bufs=2, space="PSUM") as sp, \
             tc.tile_pool(name="cp", bufs=2, space="PSUM") as cp:

            o_groups = {}

            def emit_outproj(st, half, pool, eng="v"):
                """Out-projection half-tile; results gather in 4-st o_big
                SBUF tiles, one y DMA per group (dma_start issue is ~1.2us
                of serialized SP.SEQ+HWDGE - keep the count down)."""
                g = st // 4
                if g not in o_groups:
                    o_groups[g] = ob.tile([128, 4, D], F16, tag="osb",
                                          name=f"obig{g}")
                a, b = 512 * half, 512 * half + 512
                o_ps = pool.tile([128, 512], F32, tag="o",
                                 name=f"op{st}{half}")
                for d in range(2):
                    nc.tensor.matmul(
                        o_ps[:, :],
                        lhsT=ct_sb[:, 2048 * d + 128 * st:
                                   2048 * d + 128 * (st + 1)],
                        rhs=wo_sb[:, d, a:b],
                        start=(d == 0), stop=(d == 1))
                dst = o_groups[g][:, st % 4, a:b]
                if eng == "v":
                    nc.vector.tensor_copy(dst, o_ps[:, :])
                elif eng == "s":
                    nc.scalar.copy(dst, o_ps[:, :])
                else:  # split across both engines in parallel
                    nc.vector.tensor_copy(dst[:, 0:256], o_ps[:, 0:256])
                    nc.scalar.copy(dst[:, 256:512], o_ps[:, 256:512])
                if st % 4 == 3 and half == 1:
                    nc.sync.dma_start(
                        out=y[512 * g:512 * (g + 1), :]
                        .rearrange("(s p) c -> p s c", p=128),
                        in_=o_groups[g][:, :, :])

            for ci, (h, qc) in enumerate(chunks[:-1]):
                ctx_ps = cp.tile([128, 1024], F32, tag="ctx",
                                 name=f"cx{h}{qc}")
                for kt in range(8 * (qc + 1)):
                    if (h, qc, kt) not in e_tiles:
                        emit_scores_exp(h, qc, kt, sp)
                        if (h, qc) in pending and kt in pending[(h, qc)]:
                            pending[(h, qc)].remove(kt)
                    # PE is in-order: emit READY work (ctx) first, the
                    # sp-buffer-stalled prefetch score last
                    emit_ctx(h, qc, kt, ctx_ps)
                    nxt = next((c for c in chunks[ci + 1:]
                                if pending.get(c)), None)
                    if nxt and len(e_tiles) < 30:
                        emit_scores_exp(nxt[0], nxt[1],
                                        pending[nxt].pop(0), sp)
                emit_norm(h, qc, ctx_ps)
            # all of (3,1)'s scores must exist before sp closes
            while pending[(3, 1)]:
                emit_scores_exp(3, 1, pending[(3, 1)].pop(0), sp)

        def emit_norm_half(h, qc, ctx_ps, cols):
            """Normalize a 512-col half; ctx cols [0:512] are final after
            kt = 8qc+3, so the first half runs during the tail k-tiles."""
            jh = h // 2
            hb = 64 * (h % 2)
            dr = 64 - hb
            a, b = cols
            rcp = rp.tile([128, 1024], F32, tag="rcp", name=f"rh{h}{qc}{a}")
            rcb = rp.tile([128, 1024], F32, tag="rcb", name=f"rbh{h}{qc}{a}")
            nc.vector.reciprocal(rcp[dr:dr + 1, a:b], ctx_ps[dr:dr + 1, a:b])
            if dr == 0:
                nc.gpsimd.partition_broadcast(rcb[:, a:b], rcp[0:1, a:b])
            else:
                nc.sync.dma_start(
                    out=rcb[hb:hb + 64, a:b],
                    in_=rcp[dr:dr + 1, a:b].unsqueeze(1)
                    .to_broadcast((1, 64, b - a)))
            base = 2048 * jh + 1024 * qc
            nc.vector.tensor_mul(
                ct_sb[hb:hb + 64, base + a:base + b],
                ctx_ps[hb:hb + 64, a:b], rcb[hb:hb + 64, a:b])

        # last chunk (3,1): its ctx is exp-paced leftovers; interleave the
        # qc0 out-projection halves under it; split-norm so st8-11 start
        # while kt12-15 still accumulate cols 512:1024
        with tc.tile_pool(name="cp2", bufs=1, space="PSUM") as cp2, \
             tc.tile_pool(name="op2", bufs=6, space="PSUM") as op2:
            ctx_ps = cp2.tile([128, 1024], F32, tag="ctx", name="cx31")
            op_q = [(st, half) for st in range(8) for half in range(2)]
            for kt in range(16):
                emit_ctx(3, 1, kt, ctx_ps)
                if kt == 11:
                    emit_norm_half(3, 1, ctx_ps, (0, 512))
                if op_q:
                    emit_outproj(*op_q.pop(0), pool=op2)
            emit_norm_half(3, 1, ctx_ps, (512, 1024))
            for st, half in op_q:
                emit_outproj(st, half, pool=op2)
            for st in range(8, ST):
                for half in range(2):
                    emit_outproj(st, half, pool=op2, eng="vs")

    nc.compile()
    return nc


_nc = None


def _split8(a, s):
    """s*a ~= hi + lo with hi = fp8(s*a)."""
    hi = (a * s).astype(E4)
    lo = (a * s - hi.astype(np.float32)).astype(E4)
    return hi, lo


def make_in_maps(x, W_q, W_k, W_v, W_o):
    x = np.asarray(x, dtype=np.float32)
    W_q = np.asarray(W_q, dtype=np.float32) * 0.125  # fold 1/sqrt(Dh)
    W_k = np.asarray(W_k, dtype=np.float32)
    W_v = np.asarray(W_v, dtype=np.float32)
    W_o = np.asarray(W_o, dtype=np.float32)
    mask = np.triu(np.ones((128, 128), dtype=np.float16))  # 1 where k <= q
    in_maps = []
    for c in range(N_CORES):
        b = c // 4
        g = c % 4
        sl = slice(DC * g, DC * (g + 1))
        def tile128(a):
            # [D, C] -> [128, (D//128)*C] partition-major tiling
            dd, cc = a.shape
            return np.ascontiguousarray(
                a.reshape(dd // 128, 128, cc).transpose(1, 0, 2)
                .reshape(128, -1))
        xhv, xlv = _split8(x[b].T, 1.0)
        m = {"xh": tile128(xhv), "xl": tile128(xlv), "mk": mask,
             "wo": np.ascontiguousarray(W_o[sl, :]).astype(np.float16)}
        for nm, W, s in (("q", W_q, SQ), ("k", W_k, SK), ("v", W_v, SV)):
            h_, l_ = _split8(np.ascontiguousarray(W[:, sl]), s)
            m["w" + nm + "h"] = tile128(h_)
            m["w" + nm + "l"] = tile128(l_)
        in_maps.append(m)
    return in_maps


def kernel(x, W_q, W_k, W_v, W_o, b_o):
    global _nc
    if _nc is None:
        _nc = _build()
    b_o = np.asarray(b_o, dtype=np.float32)
    in_maps = make_in_maps(x, W_q, W_k, W_v, W_o)
    res = run_bass_kernel_spmd(_nc, in_maps, list(range(N_CORES)))
    parts = [res.results[c]["y"] for c in range(N_CORES)]
    out = np.empty((B, S, D), dtype=np.float32)
    for b in range(B):
        acc = np.zeros((S, D), dtype=np.float32)
        for g in range(4):
            acc += parts[4 * b + g].astype(np.float32)
        acc += b_o
        out[b] = acc
    return out
